# revision 1
# baseline (speedup 1.0000x reference)
"""NetVLAD Trainium2 kernel.

x:(32,4096,128) f32, clusters:(64,128), clusters2:(1,64,128) ->
vlad:(32, 8192).

Math (validated against the reference, scale-rel err ~2e-6):
  L = x @ C.T                      [N, K]  per batch
  A = softmax(L, axis=K)           (no max subtraction: |L| <= ~83,
                                    exp stays in fp32 range, A <= 1)
  V = A.T @ [x | 1]                [K, D+1]  (col D = a_sum, free via
                                    ones column appended host-side)
  vlad = V[:, :D] - a_sum^2 * c2   (folded as + a_sum^2 * (-c2))

Sharding: data-parallel over batch, 4 batches per core x 8 cores.
Per core: 32 groups of 512 rows (4 chunks of 128).
"""

import os
import sys

import numpy as np

for _p in ("/opt/trn_rl_repo", "/root/.axon_site/_ro/trn_rl_repo"):
    if os.path.isdir(_p) and _p not in sys.path:
        sys.path.insert(0, _p)

import concourse.bass as bass  # noqa: E402
import concourse.tile as tile  # noqa: E402
from concourse import bacc, mybir  # noqa: E402
from concourse.bass_utils import run_bass_kernel_spmd  # noqa: E402

F32 = mybir.dt.float32
NCORES = 8
B_FULL, N, D, K = 32, 4096, 128, 64
BPC = B_FULL // NCORES  # batches per core
P = 128  # rows per chunk
CPG = 4  # chunks per group
NG = N // (P * CPG)  # groups per batch

_TRACE = False
_LAST_RESULT = None
_CACHE = {}


def _build():
    nc = bacc.Bacc("TRN2", debug=False)
    xs_e = nc.dram_tensor("xs", [BPC, NG, P, CPG, D + 1], F32, kind="ExternalInput")
    ct_e = nc.dram_tensor("ct", [D, K], F32, kind="ExternalInput")
    c2n_e = nc.dram_tensor("c2n", [K, D], F32, kind="ExternalInput")
    id_e = nc.dram_tensor("ident", [P, P], F32, kind="ExternalInput")
    y_e = nc.dram_tensor("y", [BPC, K, D], F32, kind="ExternalOutput")

    with tile.TileContext(nc) as tc:
        with (
            tc.tile_pool(name="consts", bufs=1) as cpool,
            tc.tile_pool(name="xg", bufs=3) as xpool,
            tc.tile_pool(name="xts", bufs=3) as xtpool,
            tc.tile_pool(name="ea", bufs=3) as eapool,
            tc.tile_pool(name="small", bufs=4) as spool,
            tc.tile_pool(name="ob", bufs=2) as opool,
            tc.tile_pool(name="pt", bufs=2, space="PSUM") as ptpool,
            tc.tile_pool(name="pl", bufs=2, space="PSUM") as plpool,
            tc.tile_pool(name="pv", bufs=2, space="PSUM") as pvpool,
        ):
            ct_s = cpool.tile([D, K], F32, tag="ct")
            c2n_s = cpool.tile([K, D], F32, tag="c2n")
            id_s = cpool.tile([P, P], F32, tag="id")
            nc.sync.dma_start(ct_s[:], ct_e[:])
            nc.sync.dma_start(c2n_s[:], c2n_e[:])
            nc.sync.dma_start(id_s[:], id_e[:])

            for b in range(BPC):
                vp = pvpool.tile([K, D + 1], F32, tag="vp")
                for g in range(NG):
                    xg = xpool.tile([P, CPG, D + 4], F32, tag="xg")
                    nc.sync.dma_start(xg[:, :, 0 : D + 1], xs_e[b, g])

                    xtp = ptpool.tile([P, CPG, P], F32, tag="xtp")
                    for c in range(CPG):
                        nc.tensor.transpose(xtp[:, c, :], xg[:, c, 0:D], id_s[:])
                    xts = xtpool.tile([P, CPG, P], F32, tag="xts")
                    nc.scalar.copy(xts[:], xtp[:])

                    lp = plpool.tile([P, CPG, K], F32, tag="lp")
                    for c in range(CPG):
                        nc.tensor.matmul(
                            lp[:, c, :], xts[:, c, :], ct_s[:], start=True, stop=True
                        )

                    eg = eapool.tile([P, CPG, K], F32, tag="eg")
                    nc.scalar.activation(eg[:], lp[:], mybir.ActivationFunctionType.Exp)
                    sg = spool.tile([P, CPG], F32, tag="sg")
                    nc.vector.tensor_reduce(
                        sg[:], eg[:], mybir.AxisListType.X, mybir.AluOpType.add
                    )
                    rg = spool.tile([P, CPG], F32, tag="rg")
                    nc.vector.reciprocal(rg[:], sg[:])
                    ag = eapool.tile([P, CPG, K], F32, tag="ag")
                    for c in range(CPG):
                        nc.vector.tensor_scalar_mul(
                            ag[:, c, :], eg[:, c, :], rg[:, c : c + 1]
                        )

                    for c in range(CPG):
                        nc.tensor.matmul(
                            vp[:],
                            ag[:, c, :],
                            xg[:, c, 0 : D + 1],
                            start=(g == 0 and c == 0),
                            stop=(g == NG - 1 and c == CPG - 1),
                        )

                asq = spool.tile([K, 1], F32, tag="asq")
                nc.scalar.square(asq[:], vp[:, D : D + 1])
                ob = opool.tile([K, D], F32, tag="ob")
                nc.vector.scalar_tensor_tensor(
                    ob[:],
                    c2n_s[:],
                    asq[:],
                    vp[:, 0:D],
                    mybir.AluOpType.mult,
                    mybir.AluOpType.add,
                )
                nc.sync.dma_start(y_e[b], ob[:])

    nc.compile()
    return nc


def _prep_inputs(x, clusters, clusters2):
    x = np.asarray(x, np.float32)
    ct = np.ascontiguousarray(np.asarray(clusters, np.float32).T)  # [D, K]
    c2n = np.ascontiguousarray(-np.asarray(clusters2, np.float32)[0])  # [K, D]
    ident = np.eye(P, dtype=np.float32)
    # [core, b, g, c, p, d] -> [core, b, g, p, c, d], append ones col
    xr = x.reshape(NCORES, BPC, NG, CPG, P, D).transpose(0, 1, 2, 4, 3, 5)
    ones = np.ones((NCORES, BPC, NG, P, CPG, 1), np.float32)
    xs = np.ascontiguousarray(np.concatenate([xr, ones], axis=-1))
    return [
        {"xs": xs[i], "ct": ct, "c2n": c2n, "ident": ident} for i in range(NCORES)
    ]


def kernel(x, clusters, clusters2):
    global _LAST_RESULT
    if "nc" not in _CACHE:
        _CACHE["nc"] = _build()
    nc = _CACHE["nc"]
    in_maps = _prep_inputs(x, clusters, clusters2)
    res = run_bass_kernel_spmd(nc, in_maps, list(range(NCORES)), trace=_TRACE)
    _LAST_RESULT = res
    y = np.stack([np.asarray(res.results[i]["y"]) for i in range(NCORES)])
    return y.reshape(B_FULL, K * D).astype(np.float32)
